# revision 1
# baseline (speedup 1.0000x reference)
"""Trainium2 Bass kernel for EquivariantGraphConvCheap (gnn_message_passing), v5.

v4 + flipped segment-sum (no transposes):
  - aggT[f,d] accumulated directly: per f-block fb, matmul(lhsT=T[e, fb],
    rhs=S[e, d]) -> PSUM [f,d].  Kills the PE transposes, the agg->transpose
    serial chain (~1us/slot PE stall), and the aggT vector copy.
  - xt supplied in float8e3 (fp8 lhsT, like the flipped seg matmul).
  - SWDGE warmed up by a dummy 128-row gather at program start; first slots'
    idx columns live in a separate small tile so real gathers fire early.
Everything else as v4 (e3m4 gather payload, flex lo/hi split, per-slot
gathers on 4 rotating queues, fp16 one-hot S, fp16 out, DVE bias add).
"""
import os
import numpy as np
import ml_dtypes

import concourse.bacc as bacc
import concourse.mybir as mybir
import concourse.tile as tile
from concourse import bass_utils

# ---- hardcoded problem geometry ----
N = 50000
E = 500000
H = 128
D = 4 * H
NCORES = 8
NSLOT = 49
LO_MAX = 32768
HI_BASE = 25000
HEAD_SLOTS = 4

f16 = mybir.dt.float16
f32 = mybir.dt.float32
f8 = mybir.dt.float8e3
i16 = mybir.dt.int16
np_f8 = ml_dtypes.float8_e3m4


def _hoist_extra_waits(nc, max_waits=1):
    n_fixed = 0
    for fn in nc.m.functions:
        for blk in fn.blocks:
            new_insts = []
            for ins in blk.instructions:
                si = ins.sync_info
                if si is not None and si.on_wait and len(si.on_wait) > max_waits:
                    waits = list(si.on_wait)
                    for j, w in enumerate(waits[:-max_waits]):
                        nop = mybir.InstNoOp(
                            name=f"{ins.name}-waitnop{j}", ins=[], outs=[])
                        nop.engine = ins.engine
                        nop.sync_info = mybir.SyncInfo(on_wait=[w], on_update=[])
                        new_insts.append(nop)
                    ins.sync_info = mybir.SyncInfo(
                        on_wait=waits[-max_waits:],
                        on_update=list(si.on_update or []))
                    n_fixed += 1
                new_insts.append(ins)
            blk.instructions[:] = new_insts
    return n_fixed


def build_nc(caps, hoist=True):
    """Per-core Bass program (SPMD). caps: ((cl, ch) per slot)."""
    caps = list(caps)
    nslot = len(caps)
    tot_chunks = sum(cl + ch for cl, ch in caps)
    soff = 128
    boff = soff + tot_chunks
    woff = boff + 128
    cw = woff + 8 * 128

    slot_off = []
    off = 0
    for cl, ch in caps:
        slot_off.append(off)
        off += cl + ch
    head_chunks = slot_off[HEAD_SLOTS]
    iw_head = head_chunks * 8
    iw_rest = (tot_chunks - head_chunks) * 8

    nc = bacc.Bacc("TRN2", target_bir_lowering=False, debug=False,
                   num_swdge_queues=4)
    x_lo = nc.dram_tensor("x_lo", (LO_MAX, D), f8, kind="ExternalInput")
    x_hi = nc.dram_tensor("x_hi", (N - HI_BASE, D), f8, kind="ExternalInput")
    idx_d = nc.dram_tensor("idx", (128, iw_head + iw_rest), i16,
                           kind="ExternalInput")
    cst_d = nc.dram_tensor("cst", (128, cw), f16, kind="ExternalInput")
    xt_d = nc.dram_tensor("xt", (nslot, 128, D), f8, kind="ExternalInput")
    out_d = nc.dram_tensor("out", (nslot * 128, D), f16, kind="ExternalOutput")

    with tile.TileContext(nc) as tc:
        with tc.tile_pool(name="const", bufs=1) as cp, \
             tc.tile_pool(name="gather", bufs=12) as gp, \
             tc.tile_pool(name="sel", bufs=4) as sp, \
             tc.tile_pool(name="aggps", bufs=2, space="PSUM") as aps, \
             tc.tile_pool(name="aggT", bufs=3) as atp, \
             tc.tile_pool(name="xtp", bufs=5) as xtp, \
             tc.tile_pool(name="outps", bufs=2, space="PSUM") as ops_, \
             tc.tile_pool(name="outsb", bufs=3) as osb:

            # SWDGE warmup: tiny gather of row 0 repeated, no input deps
            dummy_idx = cp.tile([128, 8], i16)
            nc.vector.memset(dummy_idx[:], 0)
            dummy_t = cp.tile([128, 1, D], f8)
            nc.gpsimd.dma_gather(
                out_ap=dummy_t[:, 0:1, :], in_ap=x_lo.ap(),
                idxs_ap=dummy_idx[:, 0:8], num_idxs=128, num_idxs_reg=128,
                elem_size=D, queue_num=0, single_packet=True)

            idx_head = cp.tile([128, iw_head], i16)
            idx_rest = cp.tile([128, iw_rest], i16)
            cst_sb = cp.tile([128, cw], f16)
            nc.sync.dma_start(out=idx_head[:],
                              in_=idx_d.ap()[:, 0:iw_head])
            nc.sync.dma_start(out=cst_sb[:, 0:boff],
                              in_=cst_d.ap()[:, 0:boff])
            nc.sync.dma_start(out=idx_rest[:],
                              in_=idx_d.ap()[:, iw_head:iw_head + iw_rest])
            nc.sync.dma_start(out=cst_sb[:, boff:cw],
                              in_=cst_d.ap()[:, boff:cw])

            iota_b = cst_sb[:, 0:128][:, None, :]

            def emit_gather(s):
                cl, ch = caps[s]
                if s < HEAD_SLOTS:
                    isb, o8 = idx_head, slot_off[s] * 8
                else:
                    isb, o8 = idx_rest, (slot_off[s] - head_chunks) * 8
                t_tile = gp.tile([128, cl + ch, D], f8)
                if cl > 0:
                    nc.gpsimd.dma_gather(
                        out_ap=t_tile[:, 0:cl, :], in_ap=x_lo.ap(),
                        idxs_ap=isb[:, o8:o8 + cl * 8],
                        num_idxs=cl * 128, num_idxs_reg=cl * 128,
                        elem_size=D, queue_num=(2 * s) % 4,
                        single_packet=True)
                if ch > 0:
                    nc.gpsimd.dma_gather(
                        out_ap=t_tile[:, cl:cl + ch, :], in_ap=x_hi.ap(),
                        idxs_ap=isb[:, o8 + cl * 8:o8 + (cl + ch) * 8],
                        num_idxs=ch * 128, num_idxs_reg=ch * 128,
                        elem_size=D, queue_num=(2 * s + 1) % 4,
                        single_packet=True)
                return t_tile

            st = {}
            for s in range(nslot + 1):
                # ---- stage A (slot s): gather, S, xt, flipped seg, copy
                if s < nslot:
                    cl, ch = caps[s]
                    cb = cl + ch
                    t_tile = emit_gather(s)
                    s_tile = sp.tile([128, cb, 128], f16)
                    nc.vector.tensor_tensor(
                        out=s_tile[:],
                        in0=iota_b.to_broadcast([128, cb, 128]),
                        in1=cst_sb[:, soff + slot_off[s]:
                                   soff + slot_off[s] + cb]
                            [:, :, None].to_broadcast([128, cb, 128]),
                        op=mybir.AluOpType.is_equal)
                    xt_sb = xtp.tile([128, D], f8)
                    nc.sync.dma_start(out=xt_sb[:], in_=xt_d.ap()[s])
                    agg_ps = aps.tile([128, D], f32, space="PSUM")
                    for fb in range(4):
                        for k in range(cb):
                            nc.tensor.matmul(
                                out=agg_ps[:, fb * 128:(fb + 1) * 128],
                                lhsT=t_tile[:, k, fb * 128:(fb + 1) * 128],
                                rhs=s_tile[:, k, :],
                                start=(k == 0), stop=(k == cb - 1))
                    aggt_sb = atp.tile([128, D], f16)
                    nc.scalar.copy(out=aggt_sb[:], in_=agg_ps[:])
                    st[s] = dict(aggt_sb=aggt_sb, xt_sb=xt_sb)

                # ---- stage B (slot s-1): out matmuls, bias, copy, store
                if 0 <= s - 1 < nslot:
                    s1 = s - 1
                    p = st.pop(s1)
                    out_ps = ops_.tile([128, D], f32, space="PSUM")
                    for c in range(4):
                        reg = out_ps[:, c * 128:(c + 1) * 128]
                        nc.tensor.matmul(
                            out=reg,
                            lhsT=p["aggt_sb"][:, c * 128:(c + 1) * 128],
                            rhs=cst_sb[:, woff + c * 128:woff + (c + 1) * 128],
                            start=True, stop=False)
                        nc.tensor.matmul(
                            out=reg,
                            lhsT=p["xt_sb"][:, c * 128:(c + 1) * 128],
                            rhs=cst_sb[:, woff + 512 + c * 128:
                                       woff + 512 + (c + 1) * 128],
                            start=False, stop=True)
                    out_sb = osb.tile([128, D], f16)
                    nc.vector.tensor_tensor(
                        out=out_sb[:, 0:128], in0=out_ps[:, 0:128],
                        in1=cst_sb[:, boff:boff + 128],
                        op=mybir.AluOpType.add)
                    nc.scalar.copy(out=out_sb[:, 128:D], in_=out_ps[:, 128:D])
                    nc.sync.dma_start(
                        out=out_d.ap()[s1 * 128:(s1 + 1) * 128, :],
                        in_=out_sb[:])

    nc.compile()
    if hoist:
        _hoist_extra_waits(nc)
    return nc


def _wrap_idx(vals, nidx):
    vp = np.zeros(nidx, dtype=np.int16)
    vp[:len(vals)] = vals
    w16 = vp.reshape(nidx // 16, 16).T
    return np.tile(w16, (8, 1))


def pack_inputs(x, edge_index, W_s_rel, W_s_root, b_s_root, W_v_rel, W_v_root):
    nblk = NCORES * NSLOT
    x = np.asarray(x, dtype=np.float32)
    xr8 = np.ascontiguousarray(x.reshape(N, D)).astype(np_f8)
    row = np.asarray(edge_index[0]).astype(np.int64)
    col = np.asarray(edge_index[1]).astype(np.int64)

    blk = row >> 7
    dslot = row & 127

    is_lo = col < HI_BASE
    is_hi = col >= LO_MAX
    is_fx = ~is_lo & ~is_hi
    ml = np.bincount(blk[is_lo], minlength=nblk)
    mh = np.bincount(blk[is_hi], minlength=nblk)
    fl = np.bincount(blk[is_fx], minlength=nblk)
    tot = ml + mh + fl

    cmin = np.zeros(nblk, dtype=np.int64)
    for b in range(nblk):
        best = 99
        for CL in range(14):
            k = min(fl[b], CL * 128 - ml[b])
            if k < 0:
                continue
            best = min(best, CL + (-(-(mh[b] + fl[b] - k) // 128)))
        cmin[b] = best

    order = np.argsort(-(cmin * 4096 + tot), kind="stable")
    assign = np.zeros((NCORES, NSLOT), dtype=np.int64)
    caps = []
    kchoice = np.zeros(nblk, dtype=np.int64)
    core_load = np.zeros(NCORES, dtype=np.int64)
    for s in range(NSLOT):
        members = order[s * 8:(s + 1) * 8]
        best = (99, 0, 0)
        for CL in range(14):
            chs = []
            ok = True
            for b in members:
                k = min(fl[b], CL * 128 - ml[b])
                if k < 0:
                    ok = False
                    break
                chs.append(-(-(mh[b] + fl[b] - k) // 128))
            if ok and CL + max(chs) < best[0]:
                best = (CL + max(chs), CL, max(chs))
        _, CL, CH = best
        caps.append((int(CL), int(CH)))
        for b in members:
            kchoice[b] = min(fl[b], CL * 128 - ml[b])
        msz = tot[members]
        free = list(range(NCORES))
        for b in members[np.argsort(-msz, kind="stable")]:
            c = min(free, key=lambda cc: core_load[cc])
            free.remove(c)
            assign[c, s] = b
            core_load[c] += tot[b]
    caps = tuple(caps)
    tot_chunks = sum(cl + ch for cl, ch in caps)
    soff = 128
    boff = soff + tot_chunks
    woff = boff + 128
    cw = woff + 8 * 128
    slot_off = np.cumsum([0] + [cl + ch for cl, ch in caps])[:NSLOT]

    half = is_hi.astype(np.int64)
    fx_idx = np.nonzero(is_fx)[0]
    fx_blk = blk[fx_idx]
    fo = np.argsort(fx_blk, kind="stable")
    fstarts = np.zeros(nblk + 1, dtype=np.int64)
    np.cumsum(np.bincount(fx_blk, minlength=nblk), out=fstarts[1:])
    ranks = np.empty(len(fx_idx), dtype=np.int64)
    ranks[fo] = np.arange(len(fx_idx)) - fstarts[fx_blk[fo]]
    half[fx_idx] = (ranks >= kchoice[fx_blk]).astype(np.int64)

    bh = blk * 2 + half
    counts = np.bincount(bh, minlength=nblk * 2)
    eorder = np.argsort(bh, kind="stable")
    col_s = col[eorder]
    dslot_s = dslot[eorder]
    starts = np.zeros(nblk * 2 + 1, dtype=np.int64)
    np.cumsum(counts, out=starts[1:])

    rels = [W_s_rel, W_v_rel, W_v_rel, W_v_rel]
    roots = [W_s_root, W_v_root, W_v_root, W_v_root]
    cst_common = np.zeros((128, cw), dtype=np.float16)
    cst_common[:, 0:128] = np.arange(128, dtype=np.float16)[None, :]
    cst_common[:, boff:boff + 128] = \
        np.asarray(b_s_root).astype(np.float16)[None, :]
    for c in range(4):
        cst_common[:, woff + c * 128:woff + (c + 1) * 128] = \
            np.asarray(rels[c]).T.astype(np.float16)
        cst_common[:, woff + 512 + c * 128:woff + 512 + (c + 1) * 128] = \
            np.asarray(roots[c]).T.astype(np.float16)

    x_lo = xr8[:LO_MAX]
    x_hi = xr8[HI_BASE:]
    x4 = x.reshape(N, 4, H)

    in_maps = []
    for c in range(NCORES):
        idx_arr = np.zeros((128, tot_chunks * 8), dtype=np.int16)
        cst = cst_common.copy()
        xt = np.zeros((NSLOT, 128, D), dtype=np_f8)
        for s in range(NSLOT):
            b = assign[c, s]
            cl, ch = caps[s]
            for hh, cap, coff in ((0, cl, slot_off[s]),
                                  (1, ch, slot_off[s] + cl)):
                g = b * 2 + hh
                e0, e1 = starts[g], starts[g + 1]
                ncnt = e1 - e0
                assert ncnt <= cap * 128, (s, b, hh, ncnt, cap)
                vals = col_s[e0:e1] - (HI_BASE if hh else 0)
                idx_arr[:, coff * 8:(coff + cap) * 8] = _wrap_idx(
                    vals.astype(np.int16), cap * 128)
                sp_ = np.full(cap * 128, -1.0, dtype=np.float16)
                sp_[:ncnt] = dslot_s[e0:e1].astype(np.float16)
                cst[:, soff + coff:soff + coff + cap] = \
                    sp_.reshape(cap, 128).T
            n0 = b * 128
            n1 = min(N, n0 + 128)
            if n1 > n0:
                xpad = np.zeros((128, 4, H), dtype=np.float32)
                xpad[:n1 - n0] = x4[n0:n1]
                xt[s] = xpad.transpose(2, 1, 0).reshape(128, D).astype(np_f8)
        in_maps.append({
            "x_lo": x_lo, "x_hi": x_hi, "idx": idx_arr, "cst": cst, "xt": xt,
        })
    meta = dict(caps=caps, assign=assign)
    return in_maps, meta


_NC_CACHE = {}
LAST_RESULTS = None


def run(x, edge_index, W_s_rel, W_s_root, b_s_root, W_v_rel, W_v_root,
        trace=False):
    global LAST_RESULTS
    in_maps, meta = pack_inputs(
        x, edge_index, W_s_rel, W_s_root, b_s_root, W_v_rel, W_v_root)
    key = meta["caps"]
    if key not in _NC_CACHE:
        _NC_CACHE[key] = build_nc(key)
    nc = _NC_CACHE[key]
    res = bass_utils.run_bass_kernel_spmd(
        nc, in_maps, core_ids=list(range(NCORES)), trace=trace)
    LAST_RESULTS = res
    assign = meta["assign"]
    out = np.zeros((N, 4, H), dtype=np.float32)
    for c in range(NCORES):
        oc = np.asarray(res.results[c]["out"], dtype=np.float32)
        for s in range(NSLOT):
            n0 = int(assign[c, s]) * 128
            n1 = min(N, n0 + 128)
            if n1 > n0:
                out[n0:n1] = oc[s * 128:s * 128 + (n1 - n0)].reshape(-1, 4, H)
    return out


def kernel(x, edge_index, W_s_rel, W_s_root, b_s_root, W_v_rel, W_v_root):
    return run(x, edge_index, W_s_rel, W_s_root, b_s_root, W_v_rel, W_v_root,
               trace=bool(os.environ.get("BASS_TRACE")))

